# revision 11
# baseline (speedup 1.0000x reference)
# Bass/Trainium2 kernel for nn_Pool2d: 3x3 max pool, stride 2, pad 1 (fill=0.0)
# Input  X [32, 192, 224, 224] f32  ->  Output [32, 192, 112, 112] f32
#
# Strategy: pure data parallel over batch (32 -> 4 per core, 8 cores).
# Per core: 768 images of 224x224, processed as 6 groups of 128 partitions.
# Max pool is separable: H-pass (3-tap stride-2 max over rows) then W-pass
# (3-tap stride-2 max over cols). Each pass = 2 tensor_tensor max ops on the
# vector engine using strided access patterns; the pad (fill 0.0) only affects
# the first output row/col, handled with a tensor_scalar max against 0.
import numpy as np

import concourse.bass as bass
import concourse.tile as tile
from concourse import bacc, mybir
from concourse.bass_utils import run_bass_kernel_spmd

P = 128          # SBUF partitions
H = W = 224
HO = WO = 112
R = 16           # output rows per tile
F32 = mybir.dt.float32

N_CORES = 8
N_BATCH = 32
N_CH = 192
NI = (N_BATCH // N_CORES) * N_CH   # images per core = 768


def pool_body(tc, y, x, ni, n_repeat=1):
    nc = tc.nc
    ng = ni // P          # image groups per core
    nt = HO // R          # H tiles per group
    with (
        tc.tile_pool(name="xin", bufs=3) as xin_pool,
        tc.tile_pool(name="pb", bufs=2) as p_pool,
        tc.tile_pool(name="hb", bufs=2) as h_pool,
        tc.tile_pool(name="qb", bufs=2) as q_pool,
        tc.tile_pool(name="yout", bufs=3) as out_pool,
    ):
        for rep in range(n_repeat):
          for g in range(ng):
            imgs = slice(g * P, (g + 1) * P)
            for t in range(nt):
                o0 = t * R
                r0 = max(0, 2 * o0 - 1)
                r1 = 2 * o0 + 2 * R
                nr = r1 - r0          # 2R for t==0, else 2R+1
                xt = xin_pool.tile([P, nr, W], F32, tag="xt",
                                   name=f"xt_{g}_{t}")
                nc.sync.dma_start(xt[:], x[imgs, r0:r1, :])

                pb = p_pool.tile([P, R, W], F32, tag="pb", name=f"pb_{g}_{t}")
                hb = h_pool.tile([P, R, W], F32, tag="hb", name=f"hb_{g}_{t}")
                qb = q_pool.tile([P, R, WO], F32, tag="qb", name=f"qb_{g}_{t}")
                ot = out_pool.tile([P, R, WO], F32, tag="ot",
                                   name=f"ot_{g}_{t}")

                # ---- H pass: hb[r] = max over the 3 input rows of window r --
                if t == 0:
                    # local row j == global row; out r <- rows 2r-1,2r,2r+1
                    nc.vector.tensor_max(
                        pb[:], xt[:, 0:2 * R - 1:2, :], xt[:, 1:2 * R:2, :])
                    nc.vector.tensor_max(
                        hb[:, 1:R, :], pb[:, 1:R, :], xt[:, 1:2 * R - 2:2, :])
                    # r==0: third tap is the zero padding row
                    nc.vector.tensor_scalar_max(
                        hb[:, 0:1, :], pb[:, 0:1, :], 0.0)
                else:
                    # local j = global - (2*o0-1); out r <- local 2r,2r+1,2r+2
                    nc.vector.tensor_max(
                        pb[:], xt[:, 1:2 * R:2, :], xt[:, 2:2 * R + 1:2, :])
                    nc.vector.tensor_max(
                        hb[:], pb[:], xt[:, 0:2 * R - 1:2, :])

                # ---- W pass: ot[:, :, i] = max over cols 2i-1,2i,2i+1 ------
                nc.vector.tensor_max(
                    qb[:], hb[:, :, 0:W - 1:2], hb[:, :, 1:W:2])
                nc.vector.tensor_max(
                    ot[:, :, 1:WO], qb[:, :, 1:WO], hb[:, :, 1:W - 2:2])
                nc.vector.tensor_scalar_max(
                    ot[:, :, 0:1], qb[:, :, 0:1], 0.0)

                nc.scalar.dma_start(y[imgs, o0:o0 + R, :], ot[:])


def pool_body_v2(tc, y, x, ni, n_repeat=1, xin_bufs=4, mid_bufs=1,
                 ot_bufs=3, out_batch=1):
    """No-halo variant: uniform 32-row input tiles; the r==0 window's top
    row is read from the previous tile's SBUF buffer (zero pad via
    tensor_scalar max for the first tile of each image). out_batch k
    accumulates k tiles of output and stores them with one DMA."""
    nc = tc.nc
    ng = ni // P
    nt = HO // R
    with (
        tc.tile_pool(name="xin", bufs=xin_bufs) as xin_pool,
        tc.tile_pool(name="pb", bufs=mid_bufs) as p_pool,
        tc.tile_pool(name="hb", bufs=mid_bufs) as h_pool,
        tc.tile_pool(name="qb", bufs=mid_bufs) as q_pool,
        tc.tile_pool(name="yout", bufs=ot_bufs) as out_pool,
    ):
        for rep in range(n_repeat):
          for g in range(ng):
            imgs = slice(g * P, (g + 1) * P)
            prev_xt = None
            ot = None
            for t in range(nt):
                o0 = t * R
                xt = xin_pool.tile([P, 2 * R, W], F32, tag="xt",
                                   name=f"xt_{rep}_{g}_{t}")
                nc.sync.dma_start(xt[:], x[imgs, 2 * o0:2 * o0 + 2 * R, :])

                pb = p_pool.tile([P, R, W], F32, tag="pb", name=f"pb_{g}_{t}")
                hb = h_pool.tile([P, R, W], F32, tag="hb", name=f"hb_{g}_{t}")
                qb = q_pool.tile([P, R, WO], F32, tag="qb", name=f"qb_{g}_{t}")
                b = t % out_batch          # position within output batch
                if b == 0:
                    nb_rows = min(out_batch, nt - t) * R
                    ot = out_pool.tile([P, nb_rows, WO], F32, tag="ot",
                                       name=f"ot_{g}_{t}")
                os_ = slice(b * R, (b + 1) * R)

                # H pass: out row r (global o0+r) <- x rows 2o-1, 2o, 2o+1
                # local: pb[r] = max(xt[2r], xt[2r+1]); third tap local 2r-1
                nc.vector.tensor_max(
                    pb[:], xt[:, 0:2 * R - 1:2, :], xt[:, 1:2 * R:2, :])
                nc.vector.tensor_max(
                    hb[:, 1:R, :], pb[:, 1:R, :], xt[:, 1:2 * R - 2:2, :])
                if t == 0:
                    nc.vector.tensor_scalar_max(
                        hb[:, 0:1, :], pb[:, 0:1, :], 0.0)
                else:
                    nc.vector.tensor_max(
                        hb[:, 0:1, :], pb[:, 0:1, :],
                        prev_xt[:, 2 * R - 1:2 * R, :])

                # W pass
                nc.vector.tensor_max(
                    qb[:], hb[:, :, 0:W - 1:2], hb[:, :, 1:W:2])
                nc.vector.tensor_max(
                    ot[:, os_, 1:WO], qb[:, :, 1:WO], hb[:, :, 1:W - 2:2])
                nc.vector.tensor_scalar_max(
                    ot[:, os_, 0:1], qb[:, :, 0:1], 0.0)

                if b == out_batch - 1 or t == nt - 1:
                    lo = (t - b) * R
                    nc.scalar.dma_start(y[imgs, lo:(t + 1) * R, :], ot[:])
                prev_xt = xt


def build_module(ni=NI, n_cores=N_CORES, n_repeat=1, chain=False, body=None):
    nc = bacc.Bacc("TRN2", target_bir_lowering=False, debug=False,
                   num_devices=n_cores)
    x = nc.dram_tensor("X", [ni, H, W], F32, kind="ExternalInput").ap()
    y = nc.dram_tensor("Y", [ni, HO, WO], F32, kind="ExternalOutput").ap()
    if chain:
        s = nc.dram_tensor("S", [128, 16], F32, kind="ExternalInput").ap()
        tt = nc.dram_tensor("T", [128, 16], F32, kind="ExternalOutput").ap()
    if body is None:
        body = pool_body_v2
    with tile.TileContext(nc) as tc:
        body(tc, y, x, ni, n_repeat=n_repeat)
        if chain:
            with tc.tile_pool(name="chain", bufs=1) as cpool:
                ct = cpool.tile([128, 16], F32, tag="ct")
                nc.sync.dma_start(ct[:], s[:])
                nc.scalar.dma_start(tt[:], ct[:])
    nc.compile()
    return nc


def kernel(X: np.ndarray) -> np.ndarray:
    X = np.ascontiguousarray(X, dtype=np.float32)
    nb = N_BATCH // N_CORES
    nc = build_module()
    in_maps = [
        {"X": np.ascontiguousarray(
            X[i * nb:(i + 1) * nb].reshape(NI, H, W))}
        for i in range(N_CORES)
    ]
    res = run_bass_kernel_spmd(nc, in_maps, core_ids=list(range(N_CORES)))
    out = np.stack([r["Y"].reshape(nb, N_CH, HO, WO) for r in res.results])
    return out.reshape(N_BATCH, N_CH, HO, WO)
